# revision 29
# baseline (speedup 1.0000x reference)
"""Trainium2 Bass kernel for the SIR-MLP network.

Computes, for each of B=65536 scenarios:
  gamma, beta, I0 = three tiny MLPs (16->10->10->10->10->1, tanh, softplus)
  then integrates the SIR ODE with RK4 over T=200 time points and returns
  the infected compartment at every time point, shape (T*B, 1) float32.

Strategy: pure data parallel over 8 NeuronCores (8192 scenarios each),
laid out [128 partitions x 64 free] (sample s = 64*p + f).

The ODE state is the scaled pair Y = (Ihat | u) with Ihat = -(beta/N)*I
and u = (beta/N)*S, in which the SIR derivative needs no extra scaling:

    D = (r*Ihat | u*Ihat)   with   r = u - gamma

Each RK4 stage is then three ops: {r, P=u*Ihat} -> D_I = r*Ihat ->
X = Y0 + c*D (packed STT), a 12-hop dependency chain per step. The
vector engine's RAW turnaround (~390ns) dominates, so the combine
accumulator A += w_s * D_s (double-buffered D tiles) and the per-chunk
output unscale (split in quarters) are placed between chain ops as
stall fillers. History chunks accumulate Ihat in place (the packed
state marches one slot per step) and are unscaled by -N/beta = 1/na
right before their DMA. Optional STREAMS=2 splits columns into two
interleaved independent chains.
"""

import os
import sys

import numpy as np

try:
    import concourse.bass as bass  # noqa: F401
except ImportError:
    for _p in ("/opt/trn_rl_repo", os.path.expanduser("~/.axon_site/_ro/trn_rl_repo")):
        if os.path.isdir(_p) and _p not in sys.path:
            sys.path.insert(0, _p)

import concourse.bass as bass
import concourse.bacc as bacc
import concourse.mybir as mybir
import concourse.tile as tile
from concourse.ap import AP
from concourse.bass_utils import run_bass_kernel_spmd

F32 = mybir.dt.float32
F16 = mybir.dt.float16
AF = mybir.ActivationFunctionType
OP = mybir.AluOpType

B = 65536
IN = 16
HL = 10
NL = 3
T = 200
NPOP = 8.6e6
NCORES = 8
BL = B // NCORES          # 8192 samples per core
P = 128                   # partitions
FW = BL // P              # 64 free columns of state per core
CHUNK = 16                # time steps per history chunk tile
MMN = 512                 # matmul moving chunk
NB = 2                    # chunks banded together per ACT op (bases 0, 64)
STREAMS = 1               # independent column streams on the vector engine

_cache = {}


def _rep_ap(base, n, w):
    """AP broadcasting a [P, w] slice to [P, n, w] (stride-0 middle dim)."""
    return AP(base.tensor, base.offset,
              [list(base.ap[0])] + [[0, n], [1, w]])


class _Stream:
    def __init__(self, cpool, hpool, wpool, c0, w, chunk_slots):
        self.c0 = c0
        self.w = w
        self.hist = [
            hpool.tile([P, s * w], F32, tag=f"hc{c0}_{k}", name=f"hc{c0}_{k}")
            for k, s in enumerate(chunk_slots)
        ]
        self.gam = None  # assigned a (gam|bet) slice by the builder
        self.na = cpool.tile([P, w], F32, tag=f"na{c0}", name=f"na{c0}")
        self.rinv = cpool.tile([P, w], F32, tag=f"ri{c0}", name=f"ri{c0}")
        self.ws = wpool.tile([P, 2 * w], F32, tag=f"ws{c0}", name=f"ws{c0}")
        self.rt = wpool.tile([P, w], F32, tag=f"rt{c0}", name=f"rt{c0}")
        self.dd = [wpool.tile([P, 2 * w], F32, tag=f"dd{c0}_{i}",
                              name=f"dd{c0}_{i}") for i in range(2)]
        self.acc = wpool.tile([P, 2 * w], F32, tag=f"ac{c0}", name=f"ac{c0}")
        self.fillers = []   # queued off-chain (op-lambda) stall fillers


def _emit_step(vec, st, t, h, chunk_slots, out, nsteps, last_slots):
    """Emit one RK4 step for stream st; returns list of op thunks in order.

    Each element is (is_chain_boundary, thunk). Fillers from st.fillers
    are spliced at chain-hop boundaries by the caller.
    """
    w = st.w
    nchunks = len(chunk_slots)
    k, j = divmod(t, CHUNK)
    ck = st.hist[k]
    y0 = ck[:, j * w : (j + 2) * w]
    i_t = ck[:, j * w : (j + 1) * w]
    u_t = ck[:, (j + 1) * w : (j + 2) * w]
    if j < CHUNK - 1 or k == nchunks - 1:
        x_dst = ck[:, (j + 1) * w : (j + 3) * w]
    else:
        x_dst = st.hist[k + 1][:, 0 : 2 * w]

    dda, ddb = st.dd[0], st.dd[1]
    ws, rt, acc, gam = st.ws, st.rt, st.acc, st.gam
    wsi, wsu = ws[:, 0:w], ws[:, w : 2 * w]
    c1, w6, w3 = 0.5 * h, h / 6.0, h / 3.0

    # Level-scheduled half-split pipeline: splitting the stage updates into
    # Xu (from P one level early) and XI halves shortens the chain to 10
    # RAW hops per step; each level's ops are mutually independent.
    ops = []
    # L1
    ops.append(lambda: vec.tensor_tensor(rt[:], u_t, gam, OP.subtract))
    ops.append(lambda: vec.tensor_tensor(dda[:, w:], u_t, i_t, OP.mult))
    ops.append("fill")
    # L2
    ops.append(lambda: vec.tensor_tensor(dda[:, 0:w], rt[:], i_t, OP.mult))
    ops.append(lambda: vec.scalar_tensor_tensor(wsu, dda[:, w:], c1, u_t,
                                                OP.mult, OP.add))      # Xu1
    ops.append("fill")
    # L3
    ops.append(lambda: vec.scalar_tensor_tensor(wsi, dda[:, 0:w], c1, i_t,
                                                OP.mult, OP.add))      # XI1
    ops.append(lambda: vec.tensor_tensor(rt[:], wsu, gam, OP.subtract))
    ops.append(lambda: vec.scalar_tensor_tensor(acc[:], dda[:], w6, y0,
                                                OP.mult, OP.add))      # A1
    # L4
    ops.append(lambda: vec.tensor_tensor(ddb[:, w:], wsu, wsi, OP.mult))
    ops.append(lambda: vec.tensor_tensor(ddb[:, 0:w], rt[:], wsi, OP.mult))
    # L5
    ops.append(lambda: vec.scalar_tensor_tensor(wsu, ddb[:, w:], c1, u_t,
                                                OP.mult, OP.add))      # Xu2
    ops.append(lambda: vec.scalar_tensor_tensor(wsi, ddb[:, 0:w], c1, i_t,
                                                OP.mult, OP.add))      # XI2
    ops.append(lambda: vec.scalar_tensor_tensor(acc[:], ddb[:], w3, acc[:],
                                                OP.mult, OP.add))      # A2
    # L6
    ops.append(lambda: vec.tensor_tensor(rt[:], wsu, gam, OP.subtract))
    ops.append(lambda: vec.tensor_tensor(dda[:, w:], wsu, wsi, OP.mult))
    ops.append("fill")
    # L7
    ops.append(lambda: vec.tensor_tensor(dda[:, 0:w], rt[:], wsi, OP.mult))
    ops.append(lambda: vec.scalar_tensor_tensor(wsu, dda[:, w:], h, u_t,
                                                OP.mult, OP.add))      # Xu3
    # L8
    ops.append(lambda: vec.scalar_tensor_tensor(wsi, dda[:, 0:w], h, i_t,
                                                OP.mult, OP.add))      # XI3
    ops.append(lambda: vec.tensor_tensor(rt[:], wsu, gam, OP.subtract))
    ops.append(lambda: vec.scalar_tensor_tensor(acc[:], dda[:], w3, acc[:],
                                                OP.mult, OP.add))      # A3
    # L9
    ops.append(lambda: vec.tensor_tensor(ddb[:, w:], wsu, wsi, OP.mult))
    ops.append(lambda: vec.tensor_tensor(ddb[:, 0:w], rt[:], wsi, OP.mult))
    # L10: u-half first so the next step's r1 (which reads only u) stalls
    # one op less on the cross-step RAW turnaround
    ops.append(lambda: vec.scalar_tensor_tensor(x_dst[:, w : 2 * w],
                                                ddb[:, w:], w6,
                                                acc[:, w : 2 * w],
                                                OP.mult, OP.add))      # Ynew_u
    ops.append(lambda: vec.scalar_tensor_tensor(x_dst[:, 0:w],
                                                ddb[:, 0:w], w6,
                                                acc[:, 0:w],
                                                OP.mult, OP.add))      # Ynew_I
    ops.append("fill")

    # chunk complete -> unscale quarters + DMA, deferred into LATER steps'
    # fill slots (must not run inside this step: it still reads slot 15/16)
    chunk_end = []
    if j == CHUNK - 1 or t == nsteps - 1:
        nslots = CHUNK if j == CHUNK - 1 else last_slots
        t0 = k * CHUNK
        qn = 4 if nslots % 4 == 0 else 1
        per = nslots // qn
        for q in range(qn):
            hv = ck[:, q * per * w : (q + 1) * per * w].rearrange(
                "p (s f) -> p s f", f=w)
            def unscale(hv=hv, per=per):
                vec.tensor_tensor(hv, hv, _rep_ap(st.rinv[:], per, w),
                                  OP.mult)
            chunk_end.append(unscale)

        def dma(ck=ck, nslots=nslots, t0=t0):
            src = ck[:, 0 : nslots * st.w].rearrange("p (s f) -> p s f",
                                                     f=st.w)
            dst = out[t0 : t0 + nslots, :].rearrange(
                "t (p f) -> p t f", p=P)[:, :, st.c0 : st.c0 + st.w]
            bass_nc_sync_dma(dst, src)
        chunk_end.append(dma)
    return ops, chunk_end


bass_nc_sync_dma = None  # patched in _build_program


def _build_program(dts):
    """Build the SPMD Bass program (one core's view). dts: list of floats."""
    global bass_nc_sync_dma
    nsteps = len(dts)
    nt = nsteps + 1  # number of output time points
    h = float(dts[0])
    assert all(abs(d - h) < 1e-6 for d in dts), "uniform time grid required"

    nc = bacc.Bacc("TRN2", target_bir_lowering=False, debug=False)
    bass_nc_sync_dma = nc.sync.dma_start

    xT = nc.declare_dram_parameter("xT", [IN, BL], F16, isOutput=False)
    wblob = nc.declare_dram_parameter("wblob", [94, 123], F16, isOutput=False)
    bblob = nc.declare_dram_parameter("bblob", [94, 5], F32, isOutput=False)
    out = nc.declare_dram_parameter("out", [nt, BL], F32, isOutput=True)

    spbuf = nc.dram_tensor("spbuf", [3, BL], F32)  # softplus outputs bounce

    # History chunk layout (slots; each stream scales by its width):
    # chunk k holds Ihat_t slots for t in [16k, 16k+16) plus one extra: the
    # packed state Y_t=(Ihat_t|u_t) spans slots [j, j+1]. Final chunk:
    # last_slots I-slots + 1 for the dead final u.
    nfull = nt // CHUNK if nt % CHUNK else nt // CHUNK - 1
    chunk_slots = [CHUNK + 1] * nfull
    last_slots = nt - nfull * CHUNK
    chunk_slots.append(last_slots + 1)

    with tile.TileContext(nc) as tc:
        with (
            tc.tile_pool(name="const", bufs=1) as cpool,
            tc.tile_pool(name="hist", bufs=1) as hpool,
            tc.tile_pool(name="mlp", bufs=3) as mpool,
            tc.tile_pool(name="psum", bufs=1, space="PSUM") as ppool,
            tc.tile_pool(name="work", bufs=1) as wpool,
        ):
            # ---------------- MLP phase (f32r matmuls) ----------------
            wb_s = cpool.tile([94, 123], F16, tag="wb")
            nc.sync.dma_start(wb_s[:], wblob[:])
            bb_s = cpool.tile([94, 5], F32, tag="bb")
            nc.sync.dma_start(bb_s[:], bblob[:])
            xt_s = cpool.tile([IN, BL], F16, tag="xt")
            nc.sync.dma_start(xt_s[:], xT[:])
            w0_s = wb_s[0:IN, 0 : 3 * HL]
            wh_s = [wb_s[:, 3 * HL + 3 * HL * l : 3 * HL + 3 * HL * (l + 1)]
                    for l in range(NL)]
            wo_s = wb_s[:, 120:123]
            b0_s = bb_s[:, 0:1]
            bh_s = [bb_s[:, 1 + l : 2 + l] for l in range(NL)]
            bo_s = bb_s[0:67, 4:5]

            ngroup = BL // (MMN * NB)
            hcur = [None] * ngroup
            for grp in range(ngroup):
                ph = ppool.tile([94, MMN], F32, tag="ph", bufs=3)
                for b_ in range(NB):
                    lo = (grp * NB + b_) * MMN
                    nc.tensor.matmul(ph[64 * b_ : 64 * b_ + 30, :], w0_s,
                                     xt_s[:, lo : lo + MMN],
                                     start=True, stop=True)
                hh = mpool.tile([94, MMN], F16, tag="h", bufs=12, name="h")
                nc.scalar.activation(hh[:], ph[:], AF.Tanh, bias=b0_s)
                hcur[grp] = hh
            for l in range(NL):
                for grp in range(ngroup):
                    ph2 = ppool.tile([94, MMN], F32, tag="ph", bufs=3)
                    for b_ in range(NB):
                        nc.tensor.matmul(ph2[64 * b_ : 64 * b_ + 30, :],
                                         wh_s[l][64 * b_ : 64 * b_ + 30, :],
                                         hcur[grp][64 * b_ : 64 * b_ + 30, :],
                                         start=True, stop=True)
                    hh = mpool.tile([94, MMN], F16, tag="h", bufs=12, name="h")
                    nc.scalar.activation(hh[:], ph2[:], AF.Tanh, bias=bh_s[l])
                    hcur[grp] = hh
            ecur = [None] * ngroup
            for grp in range(ngroup):
                po = ppool.tile([67, MMN], F32, tag="po", bufs=3)
                for b_ in range(NB):
                    nc.tensor.matmul(po[64 * b_ : 64 * b_ + 3, :],
                                     wo_s[64 * b_ : 64 * b_ + 30, :],
                                     hcur[grp][64 * b_ : 64 * b_ + 30, :],
                                     start=True, stop=True)
                e = mpool.tile([67, MMN], F32, tag="e", bufs=8, name="e")
                nc.scalar.activation(e[:], po[:], AF.Exp, bias=bo_s)
                ecur[grp] = e
            # softplus = ln(1 + exp(x)); one mega tile so the spbuf bounce
            # is 2 DMAs instead of 16
            spm = cpool.tile([67, ngroup * MMN], F32, tag="spm")
            for grp in range(ngroup):
                nc.scalar.activation(spm[:, grp * MMN : (grp + 1) * MMN],
                                     ecur[grp][:], AF.Ln, bias=1.0)
            sp_dst = spbuf.rearrange("j (g b c) -> j g b c", g=ngroup, b=NB)
            for b_, eng in zip(range(NB), (nc.sync, nc.gpsimd)):
                eng.dma_start(
                    sp_dst[:, :, b_, :],
                    spm[64 * b_ : 64 * b_ + 3, :].rearrange(
                        "j (g c) -> j g c", g=ngroup))

            # ---------------- streams + constants ----------------
            vec = nc.vector
            assert FW % STREAMS == 0
            wst = FW // STREAMS
            streams = [_Stream(cpool, hpool, wpool, s * wst, wst, chunk_slots)
                       for s in range(STREAMS)]

            # one DMA gathers gamma AND beta: spbuf rows 0,1 -> (gam|bet)
            gb = cpool.tile([P, 2 * FW], F32, tag="gb")
            nc.sync.dma_start(
                gb[:].rearrange("p (j f) -> p j f", j=2),
                spbuf[0:2].rearrange("j (p f) -> p j f", p=P))
            i_src = spbuf[2].rearrange("(p f) -> p f", f=FW)
            s0n = cpool.tile([P, FW], F32, tag="s0n")
            for st in streams:
                c0, w = st.c0, st.w
                st.gam = gb[:, c0 : c0 + w]
                bet = gb[:, FW + c0 : FW + c0 + w]
                nc.scalar.dma_start(st.hist[0][:, 0:w], i_src[:, c0 : c0 + w])
                vec.tensor_scalar_mul(st.na[:], bet, -1.0 / NPOP)
                st.fillers.append(lambda st=st: vec.reciprocal(
                    st.rinv[:], st.na[:]))   # -N/beta = 1/na; needed at t>=15
                # u_0 = na*(I_0 - N); Ihat_0 = na*I_0 (in place)
                vec.tensor_scalar(s0n[:, c0 : c0 + w], st.hist[0][:, 0:w],
                                  1.0, -NPOP, OP.mult, OP.add)
                vec.tensor_tensor(st.hist[0][:, w : 2 * w], st.na[:],
                                  s0n[:, c0 : c0 + w], OP.mult)
                vec.tensor_tensor(st.hist[0][:, 0:w], st.hist[0][:, 0:w],
                                  st.na[:], OP.mult)

            # ---------------- RK4 time stepping ----------------
            for t in range(nsteps):
                plans = [_emit_step(vec, st, t, h, chunk_slots, out, nsteps,
                                    last_slots) for st in streams]
                maxlen = max(len(p[0]) for p in plans)
                for i in range(maxlen):
                    for st, (ops, _ce) in zip(streams, plans):
                        if i >= len(ops):
                            continue
                        op = ops[i]
                        if op == "fill":
                            if STREAMS == 1 and st.fillers:
                                st.fillers.pop(0)()
                        else:
                            op()
                for st, (_ops, chunk_end) in zip(streams, plans):
                    st.fillers.extend(chunk_end)
                if STREAMS > 1:
                    for st in streams:
                        while st.fillers:
                            st.fillers.pop(0)()
            for st in streams:
                while st.fillers:
                    st.fillers.pop(0)()

    nc.compile()
    return nc


def _pack_params(W0, b0, Wh, bh, Wo, bo):
    W0p = np.ascontiguousarray(W0.transpose(2, 0, 1).reshape(IN, 3 * HL))
    b0c = np.zeros((94, 1), np.float32)
    boc = np.zeros((67, 1), np.float32)
    bhc = np.zeros((NL, 94, 1), np.float32)
    whs = np.zeros((3 * HL, 3 * HL), np.float32)
    Whp = np.zeros((NL, 94, 3 * HL), np.float32)
    for l in range(NL):
        whs[:] = 0
        for n in range(3):
            whs[n * HL : (n + 1) * HL, n * HL : (n + 1) * HL] = Wh[n, l].T
        Whp[l, 0:30] = whs
        Whp[l, 64:94] = whs
    wos = np.zeros((3 * HL, 3), np.float32)
    for n in range(3):
        wos[n * HL : (n + 1) * HL, n] = Wo[n, 0]
    Wop = np.zeros((94, 3), np.float32)
    Wop[0:30] = wos
    Wop[64:94] = wos
    for b_ in range(NB):
        b0c[64 * b_ : 64 * b_ + 30] = b0.reshape(3 * HL, 1)
        boc[64 * b_ : 64 * b_ + 3] = bo.reshape(3, 1)
        for l in range(NL):
            bhc[l, 64 * b_ : 64 * b_ + 30] = bh[:, l].reshape(3 * HL, 1)
    wblob = np.zeros((94, 123), np.float16)
    wblob[0:IN, 0 : 3 * HL] = W0p.astype(np.float16)
    for l in range(NL):
        wblob[:, 3 * HL + 3 * HL * l : 3 * HL + 3 * HL * (l + 1)] = \
            Whp[l].astype(np.float16)
    wblob[:, 120:123] = Wop.astype(np.float16)
    bblob = np.zeros((94, 5), np.float32)
    bblob[:, 0:1] = b0c
    for l in range(NL):
        bblob[:, 1 + l : 2 + l] = bhc[l]
    bblob[0:67, 4:5] = boc
    return np.ascontiguousarray(wblob), np.ascontiguousarray(bblob)


def _make_in_maps(data, W0, b0, Wh, bh, Wo, bo):
    wblob, bblob = _pack_params(
        np.asarray(W0, np.float32), np.asarray(b0, np.float32),
        np.asarray(Wh, np.float32), np.asarray(bh, np.float32),
        np.asarray(Wo, np.float32), np.asarray(bo, np.float32))
    dataT = np.ascontiguousarray(np.asarray(data, np.float16).T)  # [16, B]
    shared = {"wblob": wblob, "bblob": bblob}
    in_maps = []
    for c in range(NCORES):
        m = dict(shared)
        m["xT"] = np.ascontiguousarray(dataT[:, c * BL : (c + 1) * BL])
        in_maps.append(m)
    return in_maps


def _get_program(times):
    dts = np.diff(np.asarray(times, np.float64)).astype(np.float32)
    key = dts.tobytes()
    if key not in _cache:
        _cache[key] = _build_program([float(x) for x in dts])
    return _cache[key]


def kernel(data, times, W0, b0, Wh, bh, Wo, bo):
    nc = _get_program(times)
    in_maps = _make_in_maps(data, W0, b0, Wh, bh, Wo, bo)
    res = run_bass_kernel_spmd(nc, in_maps, list(range(NCORES)))

    nt = len(times)
    full = np.empty((nt, B), np.float32)
    for c in range(NCORES):
        full[:, c * BL : (c + 1) * BL] = res.results[c]["out"]
    return full.reshape(nt * B, 1)


def timed_run(inputs):
    """Run once with NTFF tracing enabled; returns exec_time_ns (or None)."""
    nc = _get_program(np.asarray(inputs["times"], np.float32))
    in_maps = _make_in_maps(inputs["data"], inputs["W0"], inputs["b0"],
                            inputs["Wh"], inputs["bh"], inputs["Wo"],
                            inputs["bo"])
    import shutil
    tdir = "/root/problem/trace_out"
    shutil.rmtree(tdir, ignore_errors=True)
    os.makedirs(tdir, exist_ok=True)
    res = run_bass_kernel_spmd(nc, in_maps, list(range(NCORES)), trace=True,
                               tmpdir=tdir)
    return res.exec_time_ns


# revision 30
# speedup vs baseline: 1.1976x; 1.1976x over previous
"""Trainium2 Bass kernel for the SIR-MLP network.

Computes, for each of B=65536 scenarios:
  gamma, beta, I0 = three tiny MLPs (16->10->10->10->10->1, tanh, softplus)
  then integrates the SIR ODE with RK4 over T=200 time points and returns
  the infected compartment at every time point, shape (T*B, 1) float32.

Strategy: pure data parallel over 8 NeuronCores (8192 scenarios each),
laid out [128 partitions x 64 free] (sample s = 64*p + f).

The ODE state is the scaled pair Y = (Ihat | u) with Ihat = -(beta/N)*I
and u = (beta/N)*S, in which the SIR derivative needs no extra scaling:

    D = (r*Ihat | u*Ihat)   with   r = u - gamma

Each RK4 stage is then three ops: {r, P=u*Ihat} -> D_I = r*Ihat ->
X = Y0 + c*D (packed STT), a 12-hop dependency chain per step. The
vector engine's RAW turnaround (~390ns) dominates, so the combine
accumulator A += w_s * D_s (double-buffered D tiles) and the per-chunk
output unscale (split in quarters) are placed between chain ops as
stall fillers. History chunks accumulate Ihat in place (the packed
state marches one slot per step) and are unscaled by -N/beta = 1/na
right before their DMA. Optional STREAMS=2 splits columns into two
interleaved independent chains.
"""

import os
import sys

import numpy as np

try:
    import concourse.bass as bass  # noqa: F401
except ImportError:
    for _p in ("/opt/trn_rl_repo", os.path.expanduser("~/.axon_site/_ro/trn_rl_repo")):
        if os.path.isdir(_p) and _p not in sys.path:
            sys.path.insert(0, _p)

import concourse.bass as bass
import concourse.bacc as bacc
import concourse.mybir as mybir
import concourse.tile as tile
from concourse.ap import AP
from concourse.bass_utils import run_bass_kernel_spmd

F32 = mybir.dt.float32
F16 = mybir.dt.float16
AF = mybir.ActivationFunctionType
OP = mybir.AluOpType

B = 65536
IN = 16
HL = 10
NL = 3
T = 200
NPOP = 8.6e6
NCORES = 8
BL = B // NCORES          # 8192 samples per core
P = 128                   # partitions
FW = BL // P              # 64 free columns of state per core
CHUNK = 16                # time steps per history chunk tile
MMN = 512                 # matmul moving chunk
NB = 2                    # chunks banded together per ACT op (bases 0, 64)
STREAMS = 1               # independent column streams on the vector engine

_cache = {}


def _rep_ap(base, n, w):
    """AP broadcasting a [P, w] slice to [P, n, w] (stride-0 middle dim)."""
    return AP(base.tensor, base.offset,
              [list(base.ap[0])] + [[0, n], [1, w]])


class _Stream:
    def __init__(self, cpool, hpool, wpool, c0, w, chunk_slots):
        self.c0 = c0
        self.w = w
        self.hist = [
            hpool.tile([P, s * w], F32, tag=f"hc{c0}_{k}", name=f"hc{c0}_{k}")
            for k, s in enumerate(chunk_slots)
        ]
        self.gam = None  # assigned a (gam|bet) slice by the builder
        self.na = cpool.tile([P, w], F32, tag=f"na{c0}", name=f"na{c0}")
        self.rinv = cpool.tile([P, w], F32, tag=f"ri{c0}", name=f"ri{c0}")
        self.ws = wpool.tile([P, 2 * w], F32, tag=f"ws{c0}", name=f"ws{c0}")
        self.rt = wpool.tile([P, w], F32, tag=f"rt{c0}", name=f"rt{c0}")
        self.dd = [wpool.tile([P, 2 * w], F32, tag=f"dd{c0}_{i}",
                              name=f"dd{c0}_{i}") for i in range(2)]
        self.acc = wpool.tile([P, 2 * w], F32, tag=f"ac{c0}", name=f"ac{c0}")
        self.fillers = []   # queued off-chain (op-lambda) stall fillers


def _emit_step(vec, st, t, h, chunk_slots, out, nsteps, last_slots):
    """Emit one RK4 step for stream st; returns list of op thunks in order.

    Each element is (is_chain_boundary, thunk). Fillers from st.fillers
    are spliced at chain-hop boundaries by the caller.
    """
    w = st.w
    nchunks = len(chunk_slots)
    k, j = divmod(t, CHUNK)
    ck = st.hist[k]
    y0 = ck[:, j * w : (j + 2) * w]
    i_t = ck[:, j * w : (j + 1) * w]
    u_t = ck[:, (j + 1) * w : (j + 2) * w]
    if j < CHUNK - 1 or k == nchunks - 1:
        x_dst = ck[:, (j + 1) * w : (j + 3) * w]
    else:
        x_dst = st.hist[k + 1][:, 0 : 2 * w]

    dda, ddb = st.dd[0], st.dd[1]
    ws, rt, acc, gam = st.ws, st.rt, st.acc, st.gam
    wsi, wsu = ws[:, 0:w], ws[:, w : 2 * w]
    c1, w6, w3 = 0.5 * h, h / 6.0, h / 3.0

    # Level-scheduled half-split pipeline: splitting the stage updates into
    # Xu (from P one level early) and XI halves shortens the chain to 10
    # RAW hops per step; each level's ops are mutually independent.
    ops = []
    # L1
    ops.append(lambda: vec.tensor_tensor(rt[:], u_t, gam, OP.subtract))
    ops.append(lambda: vec.tensor_tensor(dda[:, w:], u_t, i_t, OP.mult))
    ops.append("fill")
    # L2
    ops.append(lambda: vec.tensor_tensor(dda[:, 0:w], rt[:], i_t, OP.mult))
    ops.append(lambda: vec.scalar_tensor_tensor(wsu, dda[:, w:], c1, u_t,
                                                OP.mult, OP.add))      # Xu1
    ops.append("fill")
    # L3
    ops.append(lambda: vec.scalar_tensor_tensor(wsi, dda[:, 0:w], c1, i_t,
                                                OP.mult, OP.add))      # XI1
    ops.append(lambda: vec.tensor_tensor(rt[:], wsu, gam, OP.subtract))
    ops.append(lambda: vec.scalar_tensor_tensor(acc[:], dda[:], w6, y0,
                                                OP.mult, OP.add))      # A1
    # L4
    ops.append(lambda: vec.tensor_tensor(ddb[:, w:], wsu, wsi, OP.mult))
    ops.append(lambda: vec.tensor_tensor(ddb[:, 0:w], rt[:], wsi, OP.mult))
    # L5
    ops.append(lambda: vec.scalar_tensor_tensor(wsu, ddb[:, w:], c1, u_t,
                                                OP.mult, OP.add))      # Xu2
    ops.append(lambda: vec.scalar_tensor_tensor(wsi, ddb[:, 0:w], c1, i_t,
                                                OP.mult, OP.add))      # XI2
    ops.append(lambda: vec.scalar_tensor_tensor(acc[:], ddb[:], w3, acc[:],
                                                OP.mult, OP.add))      # A2
    # L6
    ops.append(lambda: vec.tensor_tensor(rt[:], wsu, gam, OP.subtract))
    ops.append(lambda: vec.tensor_tensor(dda[:, w:], wsu, wsi, OP.mult))
    ops.append("fill")
    # L7
    ops.append(lambda: vec.tensor_tensor(dda[:, 0:w], rt[:], wsi, OP.mult))
    ops.append(lambda: vec.scalar_tensor_tensor(wsu, dda[:, w:], h, u_t,
                                                OP.mult, OP.add))      # Xu3
    # L8
    ops.append(lambda: vec.scalar_tensor_tensor(wsi, dda[:, 0:w], h, i_t,
                                                OP.mult, OP.add))      # XI3
    ops.append(lambda: vec.tensor_tensor(rt[:], wsu, gam, OP.subtract))
    ops.append(lambda: vec.scalar_tensor_tensor(acc[:], dda[:], w3, acc[:],
                                                OP.mult, OP.add))      # A3
    # L9
    ops.append(lambda: vec.tensor_tensor(ddb[:, w:], wsu, wsi, OP.mult))
    ops.append(lambda: vec.tensor_tensor(ddb[:, 0:w], rt[:], wsi, OP.mult))
    # L10: u-half first so the next step's r1 (which reads only u) stalls
    # one op less on the cross-step RAW turnaround
    ops.append(lambda: vec.scalar_tensor_tensor(x_dst[:, w : 2 * w],
                                                ddb[:, w:], w6,
                                                acc[:, w : 2 * w],
                                                OP.mult, OP.add))      # Ynew_u
    ops.append(lambda: vec.scalar_tensor_tensor(x_dst[:, 0:w],
                                                ddb[:, 0:w], w6,
                                                acc[:, 0:w],
                                                OP.mult, OP.add))      # Ynew_I
    ops.append("fill")

    # chunk complete -> unscale quarters + DMA, deferred into LATER steps'
    # fill slots (must not run inside this step: it still reads slot 15/16)
    chunk_end = []
    if j == CHUNK - 1 or t == nsteps - 1:
        nslots = CHUNK if j == CHUNK - 1 else last_slots
        t0 = k * CHUNK
        qn = 4 if nslots % 4 == 0 else 1
        per = nslots // qn
        for q in range(qn):
            hv = ck[:, q * per * w : (q + 1) * per * w].rearrange(
                "p (s f) -> p s f", f=w)
            def unscale(hv=hv, per=per):
                vec.tensor_tensor(hv, hv, _rep_ap(st.rinv[:], per, w),
                                  OP.mult)
            chunk_end.append(unscale)

        def dma(ck=ck, nslots=nslots, t0=t0):
            src = ck[:, 0 : nslots * st.w].rearrange("p (s f) -> p s f",
                                                     f=st.w)
            dst = out[t0 : t0 + nslots, :].rearrange(
                "t (p f) -> p t f", p=P)[:, :, st.c0 : st.c0 + st.w]
            bass_nc_sync_dma(dst, src)
        chunk_end.append(dma)
    return ops, chunk_end


bass_nc_sync_dma = None  # patched in _build_program


def _build_program(dts):
    """Build the SPMD Bass program (one core's view). dts: list of floats."""
    global bass_nc_sync_dma
    nsteps = len(dts)
    nt = nsteps + 1  # number of output time points
    h = float(dts[0])
    assert all(abs(d - h) < 1e-6 for d in dts), "uniform time grid required"

    nc = bacc.Bacc("TRN2", target_bir_lowering=False, debug=False)
    bass_nc_sync_dma = nc.sync.dma_start

    xT = nc.declare_dram_parameter("xT", [IN, BL], F16, isOutput=False)
    wblob = nc.declare_dram_parameter("wblob", [94, 123], F16, isOutput=False)
    bblob = nc.declare_dram_parameter("bblob", [94, 5], F32, isOutput=False)
    out = nc.declare_dram_parameter("out", [nt, BL], F32, isOutput=True)

    spbuf = nc.dram_tensor("spbuf", [3, BL], F32)  # softplus outputs bounce

    # History chunk layout (slots; each stream scales by its width):
    # chunk k holds Ihat_t slots for t in [16k, 16k+16) plus one extra: the
    # packed state Y_t=(Ihat_t|u_t) spans slots [j, j+1]. Final chunk:
    # last_slots I-slots + 1 for the dead final u.
    nfull = nt // CHUNK if nt % CHUNK else nt // CHUNK - 1
    chunk_slots = [CHUNK + 1] * nfull
    last_slots = nt - nfull * CHUNK
    chunk_slots.append(last_slots + 1)

    with tile.TileContext(nc) as tc:
        with (
            tc.tile_pool(name="const", bufs=1) as cpool,
            tc.tile_pool(name="hist", bufs=1) as hpool,
            tc.tile_pool(name="mlp", bufs=3) as mpool,
            tc.tile_pool(name="psum", bufs=1, space="PSUM") as ppool,
            tc.tile_pool(name="work", bufs=1) as wpool,
        ):
            # ---------------- MLP phase (f32r matmuls) ----------------
            wb_s = cpool.tile([94, 123], F16, tag="wb")
            nc.sync.dma_start(wb_s[:], wblob[:])
            bb_s = cpool.tile([94, 5], F32, tag="bb")
            nc.sync.dma_start(bb_s[:], bblob[:])
            xt_s = cpool.tile([IN, BL], F16, tag="xt")
            nc.sync.dma_start(xt_s[:], xT[:])
            w0_s = wb_s[0:IN, 0 : 3 * HL]
            wh_s = [wb_s[:, 3 * HL + 3 * HL * l : 3 * HL + 3 * HL * (l + 1)]
                    for l in range(NL)]
            wo_s = wb_s[:, 120:123]
            b0_s = bb_s[:, 0:1]
            bh_s = [bb_s[:, 1 + l : 2 + l] for l in range(NL)]
            bo_s = bb_s[0:67, 4:5]

            # Dummy matmuls: sync each stationary tensor to PE and ramp the
            # PE out of its low P-state before the real MLP matmuls.
            dummy_ps = ppool.tile([3 * HL, 2], F32, tag="dummy")
            nc.tensor.matmul(dummy_ps[:], w0_s, w0_s[:, :2],
                             start=True, stop=True)
            for l in range(NL):
                nc.tensor.matmul(dummy_ps[:], wh_s[l][0:30, :],
                                 wh_s[l][0:30, :2], start=True, stop=True)
            nc.tensor.matmul(dummy_ps[:3, :], wo_s[0:30, :],
                             wo_s[0:30, :2], start=True, stop=True)

            ngroup = BL // (MMN * NB)
            hcur = [None] * ngroup
            for grp in range(ngroup):
                ph = ppool.tile([94, MMN], F32, tag="ph", bufs=3)
                for b_ in range(NB):
                    lo = (grp * NB + b_) * MMN
                    nc.tensor.matmul(ph[64 * b_ : 64 * b_ + 30, :], w0_s,
                                     xt_s[:, lo : lo + MMN],
                                     start=True, stop=True)
                hh = mpool.tile([94, MMN], F16, tag="h", bufs=12, name="h")
                nc.scalar.activation(hh[:], ph[:], AF.Tanh, bias=b0_s)
                hcur[grp] = hh
            for l in range(NL):
                for grp in range(ngroup):
                    ph2 = ppool.tile([94, MMN], F32, tag="ph", bufs=3)
                    for b_ in range(NB):
                        nc.tensor.matmul(ph2[64 * b_ : 64 * b_ + 30, :],
                                         wh_s[l][64 * b_ : 64 * b_ + 30, :],
                                         hcur[grp][64 * b_ : 64 * b_ + 30, :],
                                         start=True, stop=True)
                    hh = mpool.tile([94, MMN], F16, tag="h", bufs=12, name="h")
                    nc.scalar.activation(hh[:], ph2[:], AF.Tanh, bias=bh_s[l])
                    hcur[grp] = hh
            ecur = [None] * ngroup
            for grp in range(ngroup):
                po = ppool.tile([67, MMN], F32, tag="po", bufs=3)
                for b_ in range(NB):
                    nc.tensor.matmul(po[64 * b_ : 64 * b_ + 3, :],
                                     wo_s[64 * b_ : 64 * b_ + 30, :],
                                     hcur[grp][64 * b_ : 64 * b_ + 30, :],
                                     start=True, stop=True)
                e = mpool.tile([67, MMN], F32, tag="e", bufs=8, name="e")
                nc.scalar.activation(e[:], po[:], AF.Exp, bias=bo_s)
                ecur[grp] = e
            # softplus = ln(1 + exp(x)); one mega tile so the spbuf bounce
            # is 2 DMAs instead of 16
            spm = cpool.tile([67, ngroup * MMN], F32, tag="spm")
            for grp in range(ngroup):
                nc.scalar.activation(spm[:, grp * MMN : (grp + 1) * MMN],
                                     ecur[grp][:], AF.Ln, bias=1.0)
            sp_dst = spbuf.rearrange("j (g b c) -> j g b c", g=ngroup, b=NB)
            for b_, eng in zip(range(NB), (nc.sync, nc.gpsimd)):
                eng.dma_start(
                    sp_dst[:, :, b_, :],
                    spm[64 * b_ : 64 * b_ + 3, :].rearrange(
                        "j (g c) -> j g c", g=ngroup))

            # ---------------- streams + constants ----------------
            vec = nc.vector
            assert FW % STREAMS == 0
            wst = FW // STREAMS
            streams = [_Stream(cpool, hpool, wpool, s * wst, wst, chunk_slots)
                       for s in range(STREAMS)]

            # one DMA gathers gamma AND beta: spbuf rows 0,1 -> (gam|bet)
            gb = cpool.tile([P, 2 * FW], F32, tag="gb")
            nc.sync.dma_start(
                gb[:].rearrange("p (j f) -> p j f", j=2),
                spbuf[0:2].rearrange("j (p f) -> p j f", p=P))
            i_src = spbuf[2].rearrange("(p f) -> p f", f=FW)
            s0n = cpool.tile([P, FW], F32, tag="s0n")
            for st in streams:
                c0, w = st.c0, st.w
                st.gam = gb[:, c0 : c0 + w]
                bet = gb[:, FW + c0 : FW + c0 + w]
                nc.scalar.dma_start(st.hist[0][:, 0:w], i_src[:, c0 : c0 + w])
                vec.tensor_scalar_mul(st.na[:], bet, -1.0 / NPOP)
                st.fillers.append(lambda st=st: vec.reciprocal(
                    st.rinv[:], st.na[:]))   # -N/beta = 1/na; needed at t>=15
                # u_0 = na*(I_0 - N); Ihat_0 = na*I_0 (in place)
                vec.tensor_scalar(s0n[:, c0 : c0 + w], st.hist[0][:, 0:w],
                                  1.0, -NPOP, OP.mult, OP.add)
                vec.tensor_tensor(st.hist[0][:, w : 2 * w], st.na[:],
                                  s0n[:, c0 : c0 + w], OP.mult)
                vec.tensor_tensor(st.hist[0][:, 0:w], st.hist[0][:, 0:w],
                                  st.na[:], OP.mult)

            # ---------------- RK4 time stepping ----------------
            for t in range(nsteps):
                plans = [_emit_step(vec, st, t, h, chunk_slots, out, nsteps,
                                    last_slots) for st in streams]
                maxlen = max(len(p[0]) for p in plans)
                for i in range(maxlen):
                    for st, (ops, _ce) in zip(streams, plans):
                        if i >= len(ops):
                            continue
                        op = ops[i]
                        if op == "fill":
                            if STREAMS == 1 and st.fillers:
                                st.fillers.pop(0)()
                        else:
                            op()
                for st, (_ops, chunk_end) in zip(streams, plans):
                    st.fillers.extend(chunk_end)
                if STREAMS > 1:
                    for st in streams:
                        while st.fillers:
                            st.fillers.pop(0)()
            for st in streams:
                while st.fillers:
                    st.fillers.pop(0)()

    nc.compile()
    return nc


def _pack_params(W0, b0, Wh, bh, Wo, bo):
    W0p = np.ascontiguousarray(W0.transpose(2, 0, 1).reshape(IN, 3 * HL))
    b0c = np.zeros((94, 1), np.float32)
    boc = np.zeros((67, 1), np.float32)
    bhc = np.zeros((NL, 94, 1), np.float32)
    whs = np.zeros((3 * HL, 3 * HL), np.float32)
    Whp = np.zeros((NL, 94, 3 * HL), np.float32)
    for l in range(NL):
        whs[:] = 0
        for n in range(3):
            whs[n * HL : (n + 1) * HL, n * HL : (n + 1) * HL] = Wh[n, l].T
        Whp[l, 0:30] = whs
        Whp[l, 64:94] = whs
    wos = np.zeros((3 * HL, 3), np.float32)
    for n in range(3):
        wos[n * HL : (n + 1) * HL, n] = Wo[n, 0]
    Wop = np.zeros((94, 3), np.float32)
    Wop[0:30] = wos
    Wop[64:94] = wos
    for b_ in range(NB):
        b0c[64 * b_ : 64 * b_ + 30] = b0.reshape(3 * HL, 1)
        boc[64 * b_ : 64 * b_ + 3] = bo.reshape(3, 1)
        for l in range(NL):
            bhc[l, 64 * b_ : 64 * b_ + 30] = bh[:, l].reshape(3 * HL, 1)
    wblob = np.zeros((94, 123), np.float16)
    wblob[0:IN, 0 : 3 * HL] = W0p.astype(np.float16)
    for l in range(NL):
        wblob[:, 3 * HL + 3 * HL * l : 3 * HL + 3 * HL * (l + 1)] = \
            Whp[l].astype(np.float16)
    wblob[:, 120:123] = Wop.astype(np.float16)
    bblob = np.zeros((94, 5), np.float32)
    bblob[:, 0:1] = b0c
    for l in range(NL):
        bblob[:, 1 + l : 2 + l] = bhc[l]
    bblob[0:67, 4:5] = boc
    return np.ascontiguousarray(wblob), np.ascontiguousarray(bblob)


def _make_in_maps(data, W0, b0, Wh, bh, Wo, bo):
    wblob, bblob = _pack_params(
        np.asarray(W0, np.float32), np.asarray(b0, np.float32),
        np.asarray(Wh, np.float32), np.asarray(bh, np.float32),
        np.asarray(Wo, np.float32), np.asarray(bo, np.float32))
    dataT = np.ascontiguousarray(np.asarray(data, np.float16).T)  # [16, B]
    shared = {"wblob": wblob, "bblob": bblob}
    in_maps = []
    for c in range(NCORES):
        m = dict(shared)
        m["xT"] = np.ascontiguousarray(dataT[:, c * BL : (c + 1) * BL])
        in_maps.append(m)
    return in_maps


def _get_program(times):
    dts = np.diff(np.asarray(times, np.float64)).astype(np.float32)
    key = dts.tobytes()
    if key not in _cache:
        _cache[key] = _build_program([float(x) for x in dts])
    return _cache[key]


def kernel(data, times, W0, b0, Wh, bh, Wo, bo):
    nc = _get_program(times)
    in_maps = _make_in_maps(data, W0, b0, Wh, bh, Wo, bo)
    res = run_bass_kernel_spmd(nc, in_maps, list(range(NCORES)))

    nt = len(times)
    full = np.empty((nt, B), np.float32)
    for c in range(NCORES):
        full[:, c * BL : (c + 1) * BL] = res.results[c]["out"]
    return full.reshape(nt * B, 1)


def timed_run(inputs):
    """Run once with NTFF tracing enabled; returns exec_time_ns (or None)."""
    nc = _get_program(np.asarray(inputs["times"], np.float32))
    in_maps = _make_in_maps(inputs["data"], inputs["W0"], inputs["b0"],
                            inputs["Wh"], inputs["bh"], inputs["Wo"],
                            inputs["bo"])
    import shutil
    tdir = "/root/problem/trace_out"
    shutil.rmtree(tdir, ignore_errors=True)
    os.makedirs(tdir, exist_ok=True)
    res = run_bass_kernel_spmd(nc, in_maps, list(range(NCORES)), trace=True,
                               tmpdir=tdir)
    return res.exec_time_ns
